# revision 1
# baseline (speedup 1.0000x reference)
"""Single-head attention (B=4, T=4096, D_IN=1024, D_HEAD=D_OUT=64) on 8 TRN2
NeuronCores.

Sharding: core c handles batch b = c//2 and query-half h = c%2 (2048 queries),
computing K/V for the full sequence of its batch redundantly on both cores of
a pair.  Inputs are pre-transposed/permuted on the host so the device program
is identical on every core (SPMD):

  xt[c]  = x[b].T  with columns permuted so the core's own query-half comes
           first.  The s-permutation of K/V is harmless (softmax + weighted
           sum are permutation-invariant); queries come out in natural order.

Device pipeline per core (all matmuls on TensorE in fp32r rounding mode):
  A. qT = Wq.T @ x.T (own half),  [kT; vT] = [Wk|Wv].T @ x.T (full seq)
  B. v_aug[s,0:64] = v (PE-transpose of vT), v_aug[s,64] = 1.0
  C. per query-chunk: scoresT[s,tq] = kT(s-chunk) x qT;  ACT exp(scale*x)
     PSUM->SBUF;  out_augT[o,tq] += v_aug.T @ expT  (row 64 = softmax denom)
  D. PE-transpose out_augT, scale rows by reciprocal of denom, DMA out.
"""

import numpy as np

import concourse.bacc as bacc
import concourse.bass as bass
import concourse.mybir as mybir
import concourse.tile as tile
from concourse.bass_utils import run_bass_kernel_spmd

B, T, D_IN, D_HEAD, D_OUT = 4, 4096, 1024, 64, 64
N_CORES = 8
TQ = T // 2          # queries per core
ND = D_IN // 128     # contraction chunks (8)
NS = T // 128        # key/value chunks of 128 (32)
SCALE = float(1.0 / np.sqrt(np.float32(D_HEAD)))

F32 = mybir.dt.float32
F32R = mybir.dt.float32r
EXPF = mybir.ActivationFunctionType.Exp


def emit_body(nc, tc, io, dt_mm, phases="ABCD", n_iters=None):
    """Emit the per-core kernel body. io: dict of DRAM APs.

    Single scheduling window: projections for the second sequence half (and
    their DMAs) overlap the ACT-bound attention over the first half.  PSUM
    budget (8 banks): pa(2x1) + ps(2x2) + po(1x2) = 8; phase-B transposes and
    phase-D transposes borrow the 'pa'/'ps' slots respectively.
    """
    xt_d, wkv_d, wq_d = io["xt"], io["wkv"], io["wq"]
    bkv_d, bq_d, id_d, out_d = io["bkv"], io["bq"], io["ident"], io["out"]

    with (
        tc.tile_pool(name="const", bufs=1) as cpool,
        tc.tile_pool(name="xt", bufs=6) as xpool,
        tc.tile_pool(name="proj", bufs=1) as ppool,
        tc.tile_pool(name="exp", bufs=2) as epool,
        tc.tile_pool(name="outp", bufs=1) as opool,
        tc.tile_pool(name="psum", bufs=1, space="PSUM") as qpool,
    ):
        # ---- constants ----
        wkv_sb = cpool.tile([128, ND, 128], dt_mm)
        wq_sb = cpool.tile([128, ND, 64], dt_mm)
        bkv_sb = cpool.tile([128, 1], F32)
        bq_sb = cpool.tile([64, 1], F32)
        id_sb = cpool.tile([128, 128], dt_mm)
        nc.scalar.dma_start(wq_sb[:], wq_d.rearrange("(c p) h -> p c h", p=128))
        nc.scalar.dma_start(wkv_sb[:], wkv_d.rearrange("(c p) h -> p c h", p=128))
        nc.gpsimd.dma_start(bkv_sb[:], bkv_d[:])
        nc.gpsimd.dma_start(bq_sb[:], bq_d[:])
        nc.gpsimd.dma_start(id_sb[:], id_d[:])

        # per-pass tiles so consumers depend on exactly one producer each
        kvs = [ppool.tile([128, 512], dt_mm, name=f"kvs{i}") for i in range(8)]
        qts = [ppool.tile([64, 512], dt_mm, name=f"qts{i}") for i in range(4)]
        vau = [ppool.tile([128, 65], dt_mm, name=f"vau{i}") for i in range(NS)]
        osb = opool.tile([128, TQ // 128, 64], F32)
        if "D" not in phases:
            nc.vector.memset(osb[:], 0.0)

        def body():
            for s in range(NS):
                nc.gpsimd.memset(vau[s][:, 64:65].bitcast(F32), 1.0)

            # ---- phase A+B: projections, one PSUM bank per accumulation ----
            # One 2MB DMA per (half, t2) pass, alternating HWDGE rings.
            xt_tiles = {}

            def load_tile(half, t2):
                xt_t = xpool.tile([128, ND, 512], dt_mm, tag="xt",
                                  name=f"xt{half}_{t2}")
                src = xt_d[:, half * 2048 + t2 * 512:
                           half * 2048 + (t2 + 1) * 512]
                srcr = src.rearrange("(c p) t -> p c t", p=128)
                eng = nc.sync if (half * 4 + t2) % 2 == 0 else nc.scalar
                eng.dma_start(xt_t[:, 0:ND // 2, :], srcr[:, 0:ND // 2, :])
                eng.dma_start(xt_t[:, ND // 2:ND, :], srcr[:, ND // 2:ND, :])
                xt_tiles[(half, t2)] = xt_t

            def q_pass(t2):
                pq = qpool.tile([64, 512], F32, tag="pa", bufs=2, name=f"pq{t2}")
                for d in range(ND):
                    nc.tensor.matmul(pq[:], wq_sb[:, d, :],
                                     xt_tiles[(0, t2)][:, d, :],
                                     start=(d == 0), stop=(d == ND - 1))
                nc.vector.tensor_scalar_add(qts[t2][:], pq[:], bq_sb[:])

            def kv_pass(half, t2):
                pkv = qpool.tile([128, 512], F32, tag="pa", bufs=2,
                                 name=f"pkv{half}_{t2}")
                for d in range(ND):
                    nc.tensor.matmul(pkv[:], wkv_sb[:, d, :],
                                     xt_tiles[(half, t2)][:, d, :],
                                     start=(d == 0), stop=(d == ND - 1))
                i = half * 4 + t2
                nc.vector.tensor_scalar_add(kvs[i][:], pkv[:], bkv_sb[:])
                if "B" in phases:
                    for cc in range(4):
                        c = i * 4 + cc
                        pvt = qpool.tile([128, 64], dt_mm, tag="pa", bufs=2,
                                         name=f"pvt{c}")
                        nc.tensor.transpose(
                            pvt[:], kvs[i][64:128, cc * 128:(cc + 1) * 128],
                            id_sb[64:128, 64:128])
                        nc.vector.tensor_copy(vau[c][:, 0:64], pvt[:])

            pos = {}

            def attn_group(tqc, g):
                # 4 consecutive s-chunks of the attention pipeline for query
                # chunk tqc, emitted as soon as their K/V chunks exist.
                if "C" not in phases:
                    return
                if tqc not in pos:
                    pos[tqc] = qpool.tile([65, 1024], F32, tag="po", bufs=1,
                                          name=f"po{tqc}")
                po = pos[tqc]
                for s in range(4 * g, 4 * g + 4):
                    ps_t = qpool.tile([128, 1024], F32, tag="ps", bufs=2,
                                      name=f"ps{tqc}_{s}")
                    for j in range(2):
                        nc.tensor.matmul(
                            ps_t[:, j * 512:(j + 1) * 512],
                            kvs[s // 4][0:64, (s % 4) * 128:(s % 4 + 1) * 128],
                            qts[2 * tqc + j][:],
                            start=True, stop=True)
                    et = epool.tile([128, 1024], dt_mm, tag="et",
                                    name=f"et{tqc}_{s}")
                    nc.scalar.activation(et[:], ps_t[:], EXPF, scale=SCALE)
                    for j in range(2):
                        nc.tensor.matmul(
                            po[:, j * 512:(j + 1) * 512],
                            vau[s][:],
                            et[:, j * 512:(j + 1) * 512],
                            start=(s == 0), stop=(s == NS - 1))

            def finish(tqc):
                if "C" not in phases or "D" not in phases:
                    return
                oT = opool.tile([65, 1024], F32, tag="oT", bufs=2, name=f"oT{tqc}")
                nc.vector.tensor_copy(oT[:], pos[tqc][:])
                for j in range(8):
                    jj = tqc * 8 + j
                    pt = qpool.tile([128, 65], F32, tag="pa", bufs=2, name=f"pt{jj}")
                    nc.tensor.transpose(pt[:], oT[:, j * 128:(j + 1) * 128],
                                        id_sb[0:65, 0:65].bitcast(F32))
                    rec = opool.tile([128, 1], F32, tag="rec", bufs=2,
                                     name=f"rec{jj}")
                    nc.vector.reciprocal(rec[:], pt[:, 64:65])
                    nc.vector.tensor_scalar_mul(osb[:, jj, :], pt[:, 0:64], rec[:])
                odst = out_d.rearrange("(j p) o -> p j o", p=128)
                nc.sync.dma_start(odst[:, tqc * 8:(tqc + 1) * 8, :],
                                  osb[:, tqc * 8:(tqc + 1) * 8, :])

            for t2 in range(4):
                load_tile(0, t2)
            for t2 in range(4):
                load_tile(1, t2)
            # interleave attention (tqc=0) with K/V production so ACT starts
            # as soon as the first K/V chunks and qT[0:1024] exist
            q_pass(0)
            kv_pass(0, 0)
            q_pass(1)
            kv_pass(0, 1)
            attn_group(0, 0)
            q_pass(2)
            kv_pass(0, 2)
            attn_group(0, 1)
            q_pass(3)
            kv_pass(0, 3)
            attn_group(0, 2)
            for t2 in range(4):
                kv_pass(1, t2)
                attn_group(0, 3 + t2)
            attn_group(0, 7)
            for g in range(8):
                attn_group(1, g)
            finish(0)
            finish(1)

        if n_iters is None:
            body()
        else:
            with tc.For_i(0, n_iters, 1) as _i:
                body()


def build_program(dt_mm=F32R, phases="ABCD", n_iters=None):
    nc = bacc.Bacc("TRN2", target_bir_lowering=False, debug=False,
                   num_devices=N_CORES)
    io = {
        "xt": nc.dram_tensor("xt", [D_IN, T], dt_mm, kind="ExternalInput").ap(),
        "wkv": nc.dram_tensor("wkv", [D_IN, 128], dt_mm, kind="ExternalInput").ap(),
        "wq": nc.dram_tensor("wq", [D_IN, 64], dt_mm, kind="ExternalInput").ap(),
        "bkv": nc.dram_tensor("bkv", [128, 1], F32, kind="ExternalInput").ap(),
        "bq": nc.dram_tensor("bq", [64, 1], F32, kind="ExternalInput").ap(),
        "ident": nc.dram_tensor("ident", [128, 128], dt_mm, kind="ExternalInput").ap(),
        "out": nc.dram_tensor("out", [TQ, 64], F32, kind="ExternalOutput").ap(),
    }
    with tile.TileContext(nc) as tc:
        emit_body(nc, tc, io, dt_mm, phases=phases, n_iters=n_iters)
    nc.compile()
    return nc


_PROGRAM_CACHE = {}


def get_program(dt_mm=F32R):
    key = str(dt_mm)
    if key not in _PROGRAM_CACHE:
        _PROGRAM_CACHE[key] = build_program(dt_mm)
    return _PROGRAM_CACHE[key]


def make_in_maps(x, Wk, bk, Wq, bq, Wv, bv):
    x = np.asarray(x, dtype=np.float32)
    wkv = np.ascontiguousarray(np.concatenate([Wk, Wv], axis=1), dtype=np.float32)
    wq = np.ascontiguousarray(Wq, dtype=np.float32)
    bkv = np.concatenate([bk, bv]).astype(np.float32).reshape(128, 1)
    bqv = np.asarray(bq, dtype=np.float32).reshape(64, 1)
    ident = np.eye(128, dtype=np.float32)
    in_maps = []
    for c in range(N_CORES):
        b, half = c // 2, c % 2
        xb = x[b]
        own = xb[half * TQ:(half + 1) * TQ].T
        other = xb[(1 - half) * TQ:(2 - half) * TQ].T
        xt = np.ascontiguousarray(np.concatenate([own, other], axis=1))
        in_maps.append({"xt": xt, "wkv": wkv, "wq": wq, "bkv": bkv,
                        "bq": bqv, "ident": ident})
    return in_maps


def assemble(results):
    out = np.empty((B, T, D_OUT), dtype=np.float32)
    for c in range(N_CORES):
        b, half = c // 2, c % 2
        out[b, half * TQ:(half + 1) * TQ, :] = results[c]["out"]
    return out


def kernel(x, Wk, bk, Wq, bq, Wv, bv):
    nc = get_program()
    in_maps = make_in_maps(x, Wk, bk, Wq, bq, Wv, bv)
    res = run_bass_kernel_spmd(nc, in_maps, list(range(N_CORES)))
    return assemble(res.results)



# revision 29
# speedup vs baseline: 1.4016x; 1.4016x over previous
"""Single-head attention (B=4, T=4096, D_IN=1024, D_HEAD=D_OUT=64) on 8 TRN2
NeuronCores.

Sharding: core c handles batch b = c//2 and query-half h = c%2 (2048 queries),
computing K/V for the full sequence of its batch redundantly on both cores of
a pair.  Inputs are pre-transposed/permuted on the host so the device program
is identical on every core (SPMD):

  xt[c]  = x[b].T  with columns permuted so the core's own query-half comes
           first.  The s-permutation of K/V is harmless (softmax + weighted
           sum are permutation-invariant); queries come out in natural order.

Device pipeline per core (all matmuls on TensorE in fp32r rounding mode):
  A. qT = Wq.T @ x.T (own half),  [kT; vT] = [Wk|Wv].T @ x.T (full seq)
  B. v_aug[s,0:64] = v (PE-transpose of vT), v_aug[s,64] = 1.0
  C. per query-chunk: scoresT[s,tq] = kT(s-chunk) x qT;  ACT exp(scale*x)
     PSUM->SBUF;  out_augT[o,tq] += v_aug.T @ expT  (row 64 = softmax denom)
  D. PE-transpose out_augT, scale rows by reciprocal of denom, DMA out.
"""

import numpy as np

import concourse.bacc as bacc
import concourse.bass as bass
import concourse.mybir as mybir
import concourse.tile as tile
from concourse.bass_utils import run_bass_kernel_spmd

B, T, D_IN, D_HEAD, D_OUT = 4, 4096, 1024, 64, 64
N_CORES = 8
TQ = T // 2          # queries per core
ND = D_IN // 128     # contraction chunks (8)
NS = T // 128        # key/value chunks of 128 (32)
SCALE = float(1.0 / np.sqrt(np.float32(D_HEAD)))

F32 = mybir.dt.float32
F32R = mybir.dt.float32r
BF16 = mybir.dt.bfloat16
FP8 = mybir.dt.float8e4
EXPF = mybir.ActivationFunctionType.Exp
DR = mybir.MatmulPerfMode.DoubleRow


def emit_body(nc, tc, io, dt_mm, phases="ABCD", n_iters=None):
    """Emit the per-core kernel body. io: dict of DRAM APs.

    Single scheduling window: projections for the second sequence half (and
    their DMAs) overlap the ACT-bound attention over the first half.  PSUM
    budget (8 banks): pa(2x1) + ps(2x2) + po(1x2) = 8; phase-B transposes and
    phase-D transposes borrow the 'pa'/'ps' slots respectively.
    """
    xt_d, wkv_d, wq_d = io["xt"], io["wkv"], io["wq"]
    bkv_d, bq_d, id_d, out_d = io["bkv"], io["bq"], io["ident"], io["out"]

    with (
        tc.tile_pool(name="const", bufs=1) as cpool,
        tc.tile_pool(name="xt", bufs=6) as xpool,
        tc.tile_pool(name="proj", bufs=1) as ppool,
        tc.tile_pool(name="exp", bufs=2) as epool,
        tc.tile_pool(name="outp", bufs=1) as opool,
        tc.tile_pool(name="psum", bufs=1, space="PSUM") as qpool,
    ):
        # ---- constants ----
        # x and projection weights travel as bf16: halves the 16MB HBM read
        # that otherwise gates the whole front half of the kernel.
        wkv_sb = cpool.tile([128, ND, 128], BF16)
        wq_sb = cpool.tile([128, ND, 64], BF16)
        bkv_sb = cpool.tile([128, 1], F32)
        bq_sb = cpool.tile([64, 1], F32)
        id_sb = cpool.tile([128, 128], dt_mm)
        scr = cpool.tile([128, 128], F32)
        nc.scalar.dma_start(wq_sb[:], wq_d.rearrange("(c p) h -> p c h", p=128))
        nc.scalar.dma_start(wkv_sb[:], wkv_d.rearrange("(c p) h -> p c h", p=128))
        nc.gpsimd.dma_start(id_sb[:], id_d[:])
        nc.gpsimd.dma_start(bkv_sb[:], bkv_d[:])
        nc.gpsimd.dma_start(bq_sb[:], bq_d[:])

        # per-pass tiles so consumers depend on exactly one producer each
        kvs = [ppool.tile([128, 512], dt_mm, name=f"kvs{i}") for i in range(8)]
        qts = [ppool.tile([64, 512], dt_mm, name=f"qts{i}") for i in range(4)]
        # v augmented with a ones row, packed in s-chunk PAIRS for fp8
        # DoubleRow AV matmuls: vau[p][part, j, o] = v[(2p+j)*128+part, o]
        vau = [ppool.tile([128, 2, 65], FP8, name=f"vau{i}")
               for i in range(NS // 2)]
        osb = opool.tile([128, TQ // 128, 64], F32)
        if "D" not in phases:
            nc.vector.memset(osb[:], 0.0)

        def body():
            # ---- PE warm-up: the p-state clock ramps over the first ~3us
            # after the PE's first instruction.  Issue a dummy matmul that
            # depends only on a memset scratch (NOT on any DMA) so the ramp
            # clock starts at ~0.2us and every real matmul runs at 2.4 GHz.
            # The memset goes on DVE: the Pool engine is busy issuing the
            # small-constant DMAs (~1us per software-DGE issue).
            nc.vector.memset(scr[:], 0.0)
            warm = qpool.tile([128, 128], F32, tag="pa", bufs=2, name="warm")
            for w in range(2):
                nc.tensor.matmul(warm[:], scr[:], scr[:],
                                 start=True, stop=True, skip_group_check=True)

            for p in range(NS // 2):
                nc.gpsimd.memset(vau[p][:, :, 64:65], 1.0)

            # ---- x DMAs: one 1MB transfer per (half, t2) pass, issued on
            # the SP/Pool rings (never the ACT ring - ACT is the bottleneck).
            xt_tiles = {}

            def load_tile(half, t2):
                xt_t = xpool.tile([128, ND, 512], BF16, tag="xt",
                                  name=f"xt{half}_{t2}")
                src = xt_d[:, half * 2048 + t2 * 512:
                           half * 2048 + (t2 + 1) * 512]
                srcr = src.rearrange("(c p) t -> p c t", p=128)
                # all x halves on the SP ring, in consumption order: the DMA
                # engine drains transfers in issue order, so this keeps the
                # head's critical tiles (0,0) and (0,1) in front.
                eng = nc.sync
                eng.dma_start(xt_t[:, 0:ND // 2, :], srcr[:, 0:ND // 2, :])
                eng.dma_start(xt_t[:, ND // 2:ND, :], srcr[:, ND // 2:ND, :])
                xt_tiles[(half, t2)] = xt_t

            # ---- projections, one PSUM bank per accumulation ----
            def q_ops(t2):
                pq = qpool.tile([64, 512], F32, tag="pa", bufs=2,
                                name=f"pq{t2}")

                def mm(d):
                    nc.tensor.matmul(pq[:], wq_sb[:, d, :],
                                     xt_tiles[(0, t2)][:, d, :],
                                     start=(d == 0), stop=(d == ND - 1))

                mms = [(lambda d=d: mm(d)) for d in range(ND)]
                add = lambda: nc.vector.tensor_scalar_add(
                    qts[t2][:], pq[:], bq_sb[:])
                return mms, add

            def kv_ops(half, t2):
                i = half * 4 + t2
                pkv = qpool.tile([128, 512], F32, tag="pa", bufs=2,
                                 name=f"pkv{half}_{t2}")

                def mm(d):
                    nc.tensor.matmul(pkv[:], wkv_sb[:, d, :],
                                     xt_tiles[(half, t2)][:, d, :],
                                     start=(d == 0), stop=(d == ND - 1))

                def add():
                    nc.vector.tensor_scalar_add(kvs[i][:], pkv[:], bkv_sb[:])

                def vtr(cc):
                    # vau[s,o] = v[s,o] via PE transpose of the v rows of kvs
                    c = i * 4 + cc
                    pvt = qpool.tile([128, 64], dt_mm, tag="pa", bufs=2,
                                     name=f"pvt{c}")
                    nc.tensor.transpose(
                        pvt[:], kvs[i][64:128, cc * 128:(cc + 1) * 128],
                        id_sb[64:128, 64:128])
                    nc.vector.tensor_copy(vau[c // 2][:, c % 2, 0:64], pvt[:])

                mms = [(lambda d=d: mm(d)) for d in range(ND)]
                vtrs = [(lambda cc=cc: vtr(cc)) for cc in range(4)]
                return mms, add, vtrs

            pos = {}
            ets = {}

            def attn_slot(tqc, s):
                # one s-chunk of the attention pipeline: scores -> exp(fp8)
                # -> (on odd s) fp8-DoubleRow AV accumulation
                if "C" not in phases:
                    return
                if tqc not in pos:
                    # AV accumulator in [tq, o] orientation: 8 query-subtiles
                    # of 128, each [128, 65] (col 64 = softmax denominator),
                    # padded to 128 cols so the tile is exactly 2 PSUM banks
                    # and the two 2KB zero-regions align with tt 0-3 / 4-7.
                    pos[tqc] = qpool.tile([128, 8, 128], F32, tag="po",
                                          bufs=1, name=f"po{tqc}")
                po = pos[tqc]
                ps_t = qpool.tile([128, 1024], F32, tag="ps", bufs=2,
                                  name=f"ps{tqc}_{s}")
                for j in range(2):
                    nc.tensor.matmul(
                        ps_t[:, j * 512:(j + 1) * 512],
                        kvs[s // 4][0:64, (s % 4) * 128:(s % 4 + 1) * 128],
                        qts[2 * tqc + j][:],
                        start=True, stop=True)
                # exp() straight to fp8, packed in s-chunk pairs so the
                # AV matmul can run in fp8 DoubleRow mode (0.5 cyc/row)
                if s % 2 == 0:
                    ets[tqc] = epool.tile([128, 2, 1024], FP8, tag="et",
                                          name=f"et{tqc}_{s // 2}")
                et2 = ets[tqc]
                nc.scalar.activation(et2[:, s % 2, :], ps_t[:], EXPF,
                                     scale=SCALE)
                if s % 2 == 1:
                    pair = s // 2
                    for tt in range(8):
                        # out[tq, o] += et[s, tq].T @ v_aug[s, o]: the
                        # output lands pre-transposed, so no phase-D
                        # transposes are needed.  start only zeroes a
                        # 2KB region: issue it once per region (tt 0, 4).
                        nc.tensor.matmul(
                            po[:, tt, 0:65],
                            et2[:, :, tt * 128:(tt + 1) * 128],
                            vau[pair][:],
                            start=(pair == 0 and tt % 4 == 0),
                            stop=(pair == NS // 2 - 1),
                            perf_mode=DR, skip_group_check=True)

            def finish(tqc):
                if "C" not in phases or "D" not in phases:
                    return
                po = pos[tqc]
                rec = opool.tile([128, 8, 1], F32, tag="rec", bufs=2,
                                 name=f"rec{tqc}")
                nc.vector.reciprocal(rec[:], po[:, :, 64:65])
                # single broadcast multiply: out[p, tt, o] = po * rec[p, tt]
                nc.vector.tensor_tensor(
                    osb[:, tqc * 8:(tqc + 1) * 8, :], po[:, :, 0:64],
                    rec[:].to_broadcast([128, 8, 64]),
                    mybir.AluOpType.mult)
                odst = out_d.rearrange("(j p) o -> p j o", p=128)
                nc.sync.dma_start(odst[:, tqc * 8:(tqc + 1) * 8, :],
                                  osb[:, tqc * 8:(tqc + 1) * 8, :])

            for t2 in range(4):
                load_tile(0, t2)
            for t2 in range(4):
                load_tile(1, t2)

            # head: shortest critical path to the first exp().  All matmuls
            # go first in the PE stream (the vau transposes depend on DVE
            # adds and would otherwise block q1 behind a round trip).
            q0_mms, q0_add = q_ops(0)
            kv0_mms, kv0_add, kv0_vtrs = kv_ops(0, 0)
            q1_mms, q1_add = q_ops(1)
            for op in q0_mms + kv0_mms:
                op()
            q0_add()
            for op in q1_mms:
                op()
            kv0_add()
            q1_add()

            # remaining projection work drains a few micro-ops per attention
            # slot so the exp() train never starves while K/V for the later
            # s-chunks (and the tqc=1 queries) are still being produced.
            queue = list(kv0_vtrs)
            for h, t2 in [(0, 1), (0, 2), (0, 3), (1, 0), (1, 1), (1, 2),
                          (1, 3)]:
                mms, add, vtrs = kv_ops(h, t2)
                queue += mms + [add] + vtrs
            for t2 in (2, 3):
                mms, add = q_ops(t2)
                queue += mms + [add]
            drained = 0

            def drain_to(n):
                nonlocal drained
                while drained < min(n, len(queue)):
                    queue[drained]()
                    drained += 1

            for s in range(NS):
                attn_slot(0, s)
                # pass j's 13 queue ops (8 mm + add + 4 vau transposes, after
                # the 4 head vtrs) must be fully emitted before slot 4j+3
                # uses vau[2j+1]; keep a steady pace so the queue is dry
                # before tqc=1 begins.
                need = max(4 + 13 * ((s + 2) // 4),
                           (len(queue) * (s + 2)) // NS)
                drain_to(need)
            drain_to(len(queue))
            finish(0)
            for s in range(NS):
                attn_slot(1, s)
            finish(1)

        if n_iters is None:
            body()
        else:
            with tc.For_i(0, n_iters, 1) as _i:
                body()


def build_program(dt_mm=F32R, phases="ABCD", n_iters=None):
    nc = bacc.Bacc("TRN2", target_bir_lowering=False, debug=False,
                   num_devices=N_CORES)
    io = {
        "xt": nc.dram_tensor("xt", [D_IN, T], BF16, kind="ExternalInput").ap(),
        "wkv": nc.dram_tensor("wkv", [D_IN, 128], BF16, kind="ExternalInput").ap(),
        "wq": nc.dram_tensor("wq", [D_IN, 64], BF16, kind="ExternalInput").ap(),
        "bkv": nc.dram_tensor("bkv", [128, 1], F32, kind="ExternalInput").ap(),
        "bq": nc.dram_tensor("bq", [64, 1], F32, kind="ExternalInput").ap(),
        "ident": nc.dram_tensor("ident", [128, 128], dt_mm, kind="ExternalInput").ap(),
        "out": nc.dram_tensor("out", [TQ, 64], F32, kind="ExternalOutput").ap(),
    }
    with tile.TileContext(nc) as tc:
        emit_body(nc, tc, io, dt_mm, phases=phases, n_iters=n_iters)
    nc.compile()
    return nc


_PROGRAM_CACHE = {}


def get_program(dt_mm=F32R):
    key = str(dt_mm)
    if key not in _PROGRAM_CACHE:
        _PROGRAM_CACHE[key] = build_program(dt_mm)
    return _PROGRAM_CACHE[key]


def make_in_maps(x, Wk, bk, Wq, bq, Wv, bv):
    bf16 = mybir.dt.np(BF16)
    x = np.asarray(x, dtype=np.float32)
    wkv = np.ascontiguousarray(
        np.concatenate([Wk, Wv], axis=1)).astype(bf16)
    wq = np.ascontiguousarray(np.asarray(Wq, dtype=np.float32)).astype(bf16)
    bkv = np.concatenate([bk, bv]).astype(np.float32).reshape(128, 1)
    bqv = np.asarray(bq, dtype=np.float32).reshape(64, 1)
    ident = np.eye(128, dtype=np.float32)
    in_maps = []
    for c in range(N_CORES):
        b, half = c // 2, c % 2
        xb = x[b]
        own = xb[half * TQ:(half + 1) * TQ].T
        other = xb[(1 - half) * TQ:(2 - half) * TQ].T
        xt = np.ascontiguousarray(
            np.concatenate([own, other], axis=1)).astype(bf16)
        in_maps.append({"xt": xt, "wkv": wkv, "wq": wq, "bkv": bkv,
                        "bq": bqv, "ident": ident})
    return in_maps


def assemble(results):
    out = np.empty((B, T, D_OUT), dtype=np.float32)
    for c in range(N_CORES):
        b, half = c // 2, c % 2
        out[b, half * TQ:(half + 1) * TQ, :] = results[c]["out"]
    return out


def kernel(x, Wk, bk, Wq, bq, Wv, bv):
    nc = get_program()
    in_maps = make_in_maps(x, Wk, bk, Wq, bq, Wv, bv)
    res = run_bass_kernel_spmd(nc, in_maps, list(range(N_CORES)))
    return assemble(res.results)



# revision 44
# speedup vs baseline: 1.4345x; 1.0235x over previous
"""Single-head attention (B=4, T=4096, D_IN=1024, D_HEAD=D_OUT=64) on 8 TRN2
NeuronCores.

Sharding: core c handles batch b = c//2 and query-half h = c%2 (2048 queries),
computing K/V for the full sequence of its batch redundantly on both cores of
a pair.  Inputs are pre-transposed/permuted on the host so the device program
is identical on every core (SPMD):

  xt[c]  = x[b].T  with columns permuted so the core's own query-half comes
           first.  The s-permutation of K/V is harmless (softmax + weighted
           sum are permutation-invariant); queries come out in natural order.

Device pipeline per core (all matmuls on TensorE in fp32r rounding mode):
  A. qT = Wq.T @ x.T (own half),  [kT; vT] = [Wk|Wv].T @ x.T (full seq)
  B. v_aug[s,0:64] = v (PE-transpose of vT), v_aug[s,64] = 1.0
  C. per query-chunk: scoresT[s,tq] = kT(s-chunk) x qT;  ACT exp(scale*x)
     PSUM->SBUF;  out_augT[o,tq] += v_aug.T @ expT  (row 64 = softmax denom)
  D. PE-transpose out_augT, scale rows by reciprocal of denom, DMA out.
"""

import numpy as np

import concourse.bacc as bacc
import concourse.bass as bass
import concourse.mybir as mybir
import concourse.tile as tile
from concourse.bass_utils import run_bass_kernel_spmd

B, T, D_IN, D_HEAD, D_OUT = 4, 4096, 1024, 64, 64
N_CORES = 8
TQ = T // 2          # queries per core
ND = D_IN // 128     # contraction chunks (8)
NS = T // 128        # key/value chunks of 128 (32)
SCALE = float(1.0 / np.sqrt(np.float32(D_HEAD)))

F32 = mybir.dt.float32
F32R = mybir.dt.float32r
BF16 = mybir.dt.bfloat16
FP8 = mybir.dt.float8e4
EXPF = mybir.ActivationFunctionType.Exp
DR = mybir.MatmulPerfMode.DoubleRow


def emit_body(nc, tc, io, dt_mm, phases="ABCD", n_iters=None):
    """Emit the per-core kernel body. io: dict of DRAM APs.

    Single scheduling window: projections for the second sequence half (and
    their DMAs) overlap the ACT-bound attention over the first half.  PSUM
    budget (8 banks): pa(2x1) + ps(2x2) + po(1x2) = 8; phase-B transposes and
    phase-D transposes borrow the 'pa'/'ps' slots respectively.
    """
    xt_d, wkv_d, wq_d = io["xt"], io["wkv"], io["wq"]
    bkv_d, bq_d, id_d, out_d = io["bkv"], io["bq"], io["ident"], io["out"]

    with (
        tc.tile_pool(name="const", bufs=1) as cpool,
        tc.tile_pool(name="xt", bufs=6) as xpool,
        tc.tile_pool(name="proj", bufs=1) as ppool,
        tc.tile_pool(name="exp", bufs=2) as epool,
        tc.tile_pool(name="outp", bufs=1) as opool,
        tc.tile_pool(name="psum", bufs=1, space="PSUM") as qpool,
    ):
        # ---- constants ----
        # x and projection weights travel as bf16: halves the 16MB HBM read
        # that otherwise gates the whole front half of the kernel.
        wkv_sb = cpool.tile([128, ND, 128], BF16)
        wq_sb = cpool.tile([128, ND, 64], BF16)
        bkv_sb = cpool.tile([128, 1], F32)
        bq_sb = cpool.tile([64, 1], F32)
        id_sb = cpool.tile([128, 128], dt_mm)
        # PE warm-up first: the p-state clock ramps over ~3us from the PE's
        # first instruction, and instructions planned inside that window get
        # charged the slow-clock rate.  A dummy matmul fed from a memset
        # scratch (no DMA dependency; Pool's first instruction) starts the
        # ramp at ~0.4us so every real matmul is planned at 2.4 GHz.
        scr = cpool.tile([128, 128], F32)
        nc.gpsimd.memset(scr[:], 0.0)
        warm = qpool.tile([128, 128], F32, tag="pa", bufs=2, name="warm")
        for w in range(2):
            nc.tensor.matmul(warm[:], scr[:], scr[:],
                             start=True, stop=True, skip_group_check=True)

        # weights lead the SP ring so kv projections never wait on them;
        # the ACT ring issues no DMAs at all (ACT is the bottleneck engine)
        # weights arrive pre-transposed ([p, c, h] contiguous) so their DMAs
        # run at full rate; bkv/bq lead the Pool ring (needed by the first
        # adds), id last (first needed ~15us in, by the vau transposes)
        nc.gpsimd.dma_start(bkv_sb[:], bkv_d[:])
        nc.gpsimd.dma_start(bq_sb[:], bq_d[:])
        nc.gpsimd.dma_start(id_sb[:], id_d[:])

        # per-pass tiles so consumers depend on exactly one producer each
        kvs = [ppool.tile([128, 512], dt_mm, name=f"kvs{i}") for i in range(8)]
        qts = [ppool.tile([64, 512], dt_mm, name=f"qts{i}") for i in range(4)]
        # v augmented with a ones row, packed in s-chunk PAIRS for fp8
        # DoubleRow AV matmuls: vau[p][part, j, o] = v[(2p+j)*128+part, o]
        vau = [ppool.tile([128, 2, 65], FP8, name=f"vau{i}")
               for i in range(NS // 2)]
        osb = opool.tile([128, TQ // 128, 64], F32)
        if "D" not in phases:
            nc.vector.memset(osb[:], 0.0)

        def body():
            for p in range(NS // 2):
                nc.gpsimd.memset(vau[p][:, :, 64:65], 1.0)

            # ---- x DMAs: one 1MB transfer per (half, t2) pass, issued on
            # the SP/Pool rings (never the ACT ring - ACT is the bottleneck).
            xt_tiles = {}

            def load_tile(half, t2):
                xt_t = xpool.tile([128, ND, 512], BF16, tag="xt",
                                  name=f"xt{half}_{t2}")
                src = xt_d[:, half * 2048 + t2 * 512:
                           half * 2048 + (t2 + 1) * 512]
                srcr = src.rearrange("(c p) t -> p c t", p=128)
                # all x halves on the SP ring, in consumption order: the DMA
                # engine drains transfers in issue order, so this keeps the
                # head's critical tiles (0,0) and (0,1) in front.
                eng = nc.sync
                eng.dma_start(xt_t[:, 0:ND // 2, :], srcr[:, 0:ND // 2, :])
                eng.dma_start(xt_t[:, ND // 2:ND, :], srcr[:, ND // 2:ND, :])
                xt_tiles[(half, t2)] = xt_t

            # ---- projections, one PSUM bank per accumulation ----
            def q_ops(t2):
                pq = qpool.tile([64, 512], F32, tag="pa", bufs=2,
                                name=f"pq{t2}")

                def mm(d):
                    nc.tensor.matmul(pq[:], wq_sb[:, d, :],
                                     xt_tiles[(0, t2)][:, d, :],
                                     start=(d == 0), stop=(d == ND - 1))

                mms = [(lambda d=d: mm(d)) for d in range(ND)]
                add = lambda: nc.vector.tensor_scalar_add(
                    qts[t2][:], pq[:], bq_sb[:])
                return mms, add

            def kv_ops(half, t2):
                i = half * 4 + t2
                pkv = qpool.tile([128, 512], F32, tag="pa", bufs=2,
                                 name=f"pkv{half}_{t2}")

                def mm(d):
                    nc.tensor.matmul(pkv[:], wkv_sb[:, d, :],
                                     xt_tiles[(half, t2)][:, d, :],
                                     start=(d == 0), stop=(d == ND - 1))

                def add():
                    nc.vector.tensor_scalar_add(kvs[i][:], pkv[:], bkv_sb[:])

                def vtr(cc):
                    # vau[s,o] = v[s,o] via PE transpose of the v rows of kvs
                    c = i * 4 + cc
                    pvt = qpool.tile([128, 64], dt_mm, tag="pa", bufs=2,
                                     name=f"pvt{c}")
                    nc.tensor.transpose(
                        pvt[:], kvs[i][64:128, cc * 128:(cc + 1) * 128],
                        id_sb[64:128, 64:128])
                    nc.vector.tensor_copy(vau[c // 2][:, c % 2, 0:64], pvt[:])

                mms = [(lambda d=d: mm(d)) for d in range(ND)]
                vtrs = [(lambda cc=cc: vtr(cc)) for cc in range(4)]
                return mms, add, vtrs

            pos = {}
            ets = {}

            def attn_slot(tqc, s):
                # one s-chunk of the attention pipeline: scores -> exp(fp8)
                # -> (on odd s) fp8-DoubleRow AV accumulation
                if "C" not in phases:
                    return
                if tqc not in pos:
                    # AV accumulator in [tq, o] orientation: 8 query-subtiles
                    # of 128, each [128, 65] (col 64 = softmax denominator),
                    # padded to 128 cols so the tile is exactly 2 PSUM banks
                    # and the two 2KB zero-regions align with tt 0-3 / 4-7.
                    pos[tqc] = qpool.tile([128, 8, 128], F32, tag="po",
                                          bufs=1, name=f"po{tqc}")
                po = pos[tqc]
                ps_t = qpool.tile([128, 1024], F32, tag="ps", bufs=2,
                                  name=f"ps{tqc}_{s}")
                for j in range(2):
                    nc.tensor.matmul(
                        ps_t[:, j * 512:(j + 1) * 512],
                        kvs[s // 4][0:64, (s % 4) * 128:(s % 4 + 1) * 128],
                        qts[2 * tqc + j][:],
                        start=True, stop=True)
                # exp() straight to fp8, packed in s-chunk pairs so the
                # AV matmul can run in fp8 DoubleRow mode (0.5 cyc/row)
                if s % 2 == 0:
                    ets[tqc] = epool.tile([128, 2, 1024], FP8, tag="et",
                                          name=f"et{tqc}_{s // 2}")
                et2 = ets[tqc]
                nc.scalar.activation(et2[:, s % 2, :], ps_t[:], EXPF,
                                     scale=SCALE)
                if s % 2 == 1:
                    pair = s // 2
                    for tt in range(8):
                        # out[tq, o] += et[s, tq].T @ v_aug[s, o]: the
                        # output lands pre-transposed, so no phase-D
                        # transposes are needed.  start only zeroes a
                        # 2KB region: issue it once per region (tt 0, 4).
                        nc.tensor.matmul(
                            po[:, tt, 0:65],
                            et2[:, :, tt * 128:(tt + 1) * 128],
                            vau[pair][:],
                            start=(pair == 0 and tt % 4 == 0),
                            stop=(pair == NS // 2 - 1),
                            perf_mode=DR, skip_group_check=True)

            def finish(tqc):
                if "C" not in phases or "D" not in phases:
                    return
                po = pos[tqc]
                rec = opool.tile([128, 8, 1], F32, tag="rec", bufs=2,
                                 name=f"rec{tqc}")
                nc.vector.reciprocal(rec[:], po[:, :, 64:65])
                odst = out_d.rearrange("(j p) o -> p j o", p=128)
                # broadcast multiply out[p, tt, o] = po * rec[p, tt], in two
                # halves so the first output DMA overlaps the second multiply
                for h in range(2):
                    t0 = 4 * h
                    j0 = tqc * 8 + t0
                    nc.vector.tensor_tensor(
                        osb[:, j0:j0 + 4, :], po[:, t0:t0 + 4, 0:64],
                        rec[:, t0:t0 + 4, :].to_broadcast([128, 4, 64]),
                        mybir.AluOpType.mult)
                    nc.sync.dma_start(odst[:, j0:j0 + 4, :],
                                      osb[:, j0:j0 + 4, :])

            # SP-ring issue order == DMA drain order: wq first (tiny, gates
            # q0), then xt(0,0), then wkv (gates kv00, hidden behind q0's
            # compute), then the rest in consumption order.
            nc.sync.dma_start(wq_sb[:], wq_d[:])
            load_tile(0, 0)
            nc.sync.dma_start(wkv_sb[:], wkv_d[:])
            for t2 in range(1, 4):
                load_tile(0, t2)
            for t2 in range(4):
                load_tile(1, t2)

            # head: shortest critical path to the first exp().  All matmuls
            # go first in the PE stream (the vau transposes depend on DVE
            # adds and would otherwise block q1 behind a round trip).
            q0_mms, q0_add = q_ops(0)
            kv0_mms, kv0_add, kv0_vtrs = kv_ops(0, 0)
            q1_mms, q1_add = q_ops(1)
            for op in q0_mms + kv0_mms:
                op()
            q0_add()
            for op in q1_mms:
                op()
            kv0_add()
            q1_add()

            # remaining projection work drains a few micro-ops per attention
            # slot so the exp() train never starves while K/V for the later
            # s-chunks (and the tqc=1 queries) are still being produced.
            queue = list(kv0_vtrs)
            for h, t2 in [(0, 1), (0, 2), (0, 3), (1, 0), (1, 1), (1, 2),
                          (1, 3)]:
                mms, add, vtrs = kv_ops(h, t2)
                queue += mms + [add] + vtrs
            for t2 in (2, 3):
                mms, add = q_ops(t2)
                queue += mms + [add]
            drained = 0

            def drain_to(n):
                nonlocal drained
                while drained < min(n, len(queue)):
                    queue[drained]()
                    drained += 1

            for s in range(NS):
                attn_slot(0, s)
                # pass j's 13 queue ops (8 mm + add + 4 vau transposes, after
                # the 4 head vtrs) must be fully emitted before slot 4j+3
                # uses vau[2j+1]; keep a steady pace so the queue is dry
                # before tqc=1 begins.
                need = max(4 + 13 * ((s + 2) // 4),
                           (len(queue) * (s + 2)) // NS)
                drain_to(need)
            drain_to(len(queue))
            finish(0)
            for s in range(NS):
                attn_slot(1, s)
            finish(1)

        if n_iters is None:
            body()
        else:
            with tc.For_i(0, n_iters, 1) as _i:
                body()


def build_program(dt_mm=F32R, phases="ABCD", n_iters=None):
    nc = bacc.Bacc("TRN2", target_bir_lowering=False, debug=False,
                   num_devices=N_CORES)
    io = {
        "xt": nc.dram_tensor("xt", [D_IN, T], BF16, kind="ExternalInput").ap(),
        "wkv": nc.dram_tensor("wkv", [128, D_IN // 128, 128], BF16,
                              kind="ExternalInput").ap(),
        "wq": nc.dram_tensor("wq", [128, D_IN // 128, 64], BF16,
                             kind="ExternalInput").ap(),
        "bkv": nc.dram_tensor("bkv", [128, 1], F32, kind="ExternalInput").ap(),
        "bq": nc.dram_tensor("bq", [64, 1], F32, kind="ExternalInput").ap(),
        "ident": nc.dram_tensor("ident", [128, 128], dt_mm, kind="ExternalInput").ap(),
        "out": nc.dram_tensor("out", [TQ, 64], F32, kind="ExternalOutput").ap(),
    }
    with tile.TileContext(nc) as tc:
        emit_body(nc, tc, io, dt_mm, phases=phases, n_iters=n_iters)
    nc.compile()
    return nc


_PROGRAM_CACHE = {}


def get_program(dt_mm=F32R):
    key = str(dt_mm)
    if key not in _PROGRAM_CACHE:
        _PROGRAM_CACHE[key] = build_program(dt_mm)
    return _PROGRAM_CACHE[key]


def make_in_maps(x, Wk, bk, Wq, bq, Wv, bv):
    bf16 = mybir.dt.np(BF16)
    x = np.asarray(x, dtype=np.float32)
    # weights pre-transposed to the SBUF layout [p, c, h] (p = partition
    # within d_in chunk, c = chunk) so the weight DMA is fully contiguous
    wkv = np.ascontiguousarray(
        np.concatenate([Wk, Wv], axis=1).reshape(ND, 128, 128)
        .transpose(1, 0, 2)).astype(bf16)
    wq = np.ascontiguousarray(
        np.asarray(Wq, dtype=np.float32).reshape(ND, 128, 64)
        .transpose(1, 0, 2)).astype(bf16)
    bkv = np.concatenate([bk, bv]).astype(np.float32).reshape(128, 1)
    bqv = np.asarray(bq, dtype=np.float32).reshape(64, 1)
    ident = np.eye(128, dtype=np.float32)
    in_maps = []
    for c in range(N_CORES):
        b, half = c // 2, c % 2
        xb = x[b]
        own = xb[half * TQ:(half + 1) * TQ].T
        other = xb[(1 - half) * TQ:(2 - half) * TQ].T
        xt = np.ascontiguousarray(
            np.concatenate([own, other], axis=1)).astype(bf16)
        in_maps.append({"xt": xt, "wkv": wkv, "wq": wq, "bkv": bkv,
                        "bq": bqv, "ident": ident})
    return in_maps


def assemble(results):
    out = np.empty((B, T, D_OUT), dtype=np.float32)
    for c in range(N_CORES):
        b, half = c // 2, c % 2
        out[b, half * TQ:(half + 1) * TQ, :] = results[c]["out"]
    return out


def kernel(x, Wk, bk, Wq, bq, Wv, bv):
    nc = get_program()
    in_maps = make_in_maps(x, Wk, bk, Wq, bq, Wv, bv)
    res = run_bass_kernel_spmd(nc, in_maps, list(range(N_CORES)))
    return assemble(res.results)

